# revision 22
# baseline (speedup 1.0000x reference)
"""HSRnn (2-layer spiking RNN) + LM head, Trainium2 Bass kernel, 8 cores.

Sharding: core c handles batch sample b = c//2 for the recurrent layers
(data-parallel over batch, duplicated per vocab half) and vocab half
v = c%2 of the lm_head projection (tensor-parallel over vocab).

Math (per layer l, per timestep t):
    V_t = alpha * V_{t-1} + (in_t @ w_in + b_in)     alpha = sigmoid(-softplus-exp(tau)) = 1/(1+e^tau)
    s_t = (V_t > thr)
    D_t = (1-2 s_t) * D_{t-1} + s_t                  (= D_{t-1} XOR s_t)
    out_t = tanh([V_t, D_t] @ w_out + b_out)
    logits = out1 @ w_head + b_head

Both recurrences are first-order linear scans -> hardware tensor_tensor_scan
along the free (time) axis with hidden units on partitions.  All matmuls run
in float32r (TF32-like fast fp32 mode).
"""
import os
import numpy as np
import concourse.bass as bass
import concourse.mybir as mybir
from concourse import tile
from concourse.bacc import Bacc
from concourse.bass_utils import run_bass_kernel_spmd

F32 = mybir.dt.float32
F32R = mybir.dt.float32r
AF = mybir.ActivationFunctionType
OP = mybir.AluOpType

B, S, I, H, V = 4, 1024, 512, 1024, 32000
NCORE = 8
VSH = V // 2            # vocab slice per core (2-way vocab split)
T = S                   # tokens per core (one sample)
NCH = 2                 # token n-chunks for matmul phases (512 each)
TCH = T // NCH
VCH = 500               # lm_head vocab chunk
NVC = VSH // VCH        # 32
KI = I // 128           # 4
KH = H // 128           # 8
K2H = 2 * H // 128      # 16

_cached = {}


def _build_program():
    nc = Bacc()
    xT = nc.declare_dram_parameter("xT", [I, T], F32, isOutput=False)
    w_in_0 = nc.declare_dram_parameter("w_in_0", [I, H], F32, isOutput=False)
    w_out_0 = nc.declare_dram_parameter("w_out_0", [2 * H, H], F32, isOutput=False)
    w_in_1 = nc.declare_dram_parameter("w_in_1", [H, H], F32, isOutput=False)
    w_out_1 = nc.declare_dram_parameter("w_out_1", [2 * H, H], F32R, isOutput=False)
    whead = nc.declare_dram_parameter("whead", [H, VSH], F32R, isOutput=False)
    bhead = nc.declare_dram_parameter("bhead", [1, VSH], F32, isOutput=False)
    # per-partition packed params [128, KH]: col j = vec[j*128:(j+1)*128]
    pp_names = ["alpha0", "thr0", "bin0", "bout0", "alpha1", "thr1", "bin1", "bout1"]
    pps = {n: nc.declare_dram_parameter(n, [128, KH], F32, isOutput=False)
           for n in pp_names}
    logits = nc.declare_dram_parameter("logits", [T, VSH], F32, isOutput=True)
    dbg = os.environ.get("KDBG") == "1"
    if dbg:
        dbg_names = ["P0d", "V0d", "D0d", "O0d", "P1d", "V1d", "D1d", "O1d"]
        dbg_t = {n: nc.declare_dram_parameter(n, [H, T], F32, isOutput=True)
                 for n in dbg_names}
        dbg_k = {n: t.rearrange("(k p) n -> k p n", p=128)
                 for n, t in dbg_t.items()}

    xT_k = xT.rearrange("(k p) n -> k p n", p=128)
    win0_k = w_in_0.rearrange("(k p) n -> k p n", p=128)
    wout0_k = w_out_0.rearrange("(k p) n -> k p n", p=128)
    win1_k = w_in_1.rearrange("(k p) n -> k p n", p=128)
    wout1_k = w_out_1.rearrange("(k p) n -> k p n", p=128)
    whead_k = whead.rearrange("(k p) n -> k p n", p=128)

    with tile.TileContext(nc) as tc:
        with tc.tile_pool(name="const", bufs=1) as cpool, \
             tc.tile_pool(name="xp", bufs=1) as xpool, \
             tc.tile_pool(name="wk", bufs=6) as wkpool, \
             tc.tile_pool(name="acts", bufs=8) as apool, \
             tc.tile_pool(name="scan", bufs=2) as spool, \
             tc.tile_pool(name="ps", bufs=8, space="PSUM") as pspool:

            pp_sb = {n: cpool.tile_from(pps[n][:], name=f"pp_{n}") for n in pp_names}

            # resident x^T k-tiles
            x_tiles = []
            for k in range(KI):
                xt = xpool.tile([128, T], F32, name=f"x{k}", tag="x", bufs=KI)
                nc.sync.dma_start(xt[:], xT_k[k])
                x_tiles.append(xt)

            def mm_phase(w_kview, n_ktiles, rhs_tiles, evict, wdt=F32):
                """out[m-tile][:, nch] = evict(sum_k w[k][:,m].T @ rhs[k][:,nch]).
                k-outer / m-inner with all 8 psum banks live per n-chunk;
                weight k-tiles streamed (re-loaded per n-chunk).
                evict(m, nsl, psum) -> writes destination tile slice."""
                for n in range(NCH):
                    nsl = slice(n * TCH, (n + 1) * TCH)
                    psums = [pspool.tile([128, TCH], F32, name=f"ps{n}_{m}", tag="p")
                             for m in range(KH)]
                    for k in range(n_ktiles):
                        wt = wkpool.tile([128, H], wdt, name=f"w{n}_{k}", tag="wk")
                        nc.sync.dma_start(wt[:], w_kview[k])
                        for m in range(KH):
                            nc.tensor.matmul(
                                psums[m][:],
                                wt[:, m * 128:(m + 1) * 128],
                                rhs_tiles[k][:, nsl],
                                start=(k == 0), stop=(k == n_ktiles - 1))
                    for m in range(KH):
                        evict(m, nsl, psums[m])

            def scan_phase(v_tiles, d_tiles, alpha_pp, thr_pp, pd=None, vd=None, dd=None,
                           vr_tiles=None):
                """per H-tile j: V-scan (in-place), spikes, D-scan."""
                for j in range(KH):
                    vj = v_tiles[j]
                    if dbg and pd is not None:
                        nc.sync.dma_start(pd[j], vj[:])
                    abc = spool.tile([128, T], F32, name=f"abc{j}", tag="abc", bufs=1)
                    nc.scalar.activation(abc[:], vj[:], AF.Identity,
                                         bias=alpha_pp[:, j:j + 1], scale=0.0)
                    nc.vector.tensor_tensor_scan(vj[:], abc[:], vj[:], 0.0,
                                                 OP.mult, OP.add)
                    sj = spool.tile([128, T], F32, name=f"s{j}", tag="s")
                    nc.vector.tensor_scalar(sj[:], vj[:],
                                            thr_pp[:, j:j + 1], None, OP.is_gt)
                    a0 = spool.tile([128, T], F32, name=f"a0_{j}", tag="a0", bufs=1)
                    nc.scalar.activation(a0[:], sj[:], AF.Identity,
                                         scale=-2.0, bias=1.0)
                    nc.vector.tensor_tensor_scan(d_tiles[j][:], a0[:],
                                                 sj[:], 0.0,
                                                 OP.mult, OP.add)
                    if dbg and vd is not None:
                        nc.sync.dma_start(vd[j], vj[:])
                        nc.sync.dma_start(dd[j], d_tiles[j][:].bitcast(F32))
                    if vr_tiles is not None:
                        # round V -> f32r copy for the following f32r matmul
                        nc.vector.tensor_copy(vr_tiles[j][:], vj[:])

            # ---------------- layer 0 ----------------
            v0 = [apool.tile([128, T], F32, name=f"v0_{j}", tag="v") for j in range(KH)]
            d0 = [apool.tile([128, T], F32, name=f"d0_{j}", tag="d") for j in range(KH)]

            def ev_p0(m, nsl, psum):
                nc.scalar.activation(v0[m][:, nsl], psum[:], AF.Identity,
                                     bias=pp_sb["bin0"][:, m:m + 1])
            mm_phase(win0_k, KI, x_tiles, ev_p0)
            scan_phase(v0, d0, pp_sb["alpha0"], pp_sb["thr0"],
                       *((dbg_k["P0d"], dbg_k["V0d"], dbg_k["D0d"]) if dbg else (None, None, None)))

            out0 = [apool.tile([128, T], F32, name=f"o0_{j}", tag="o") for j in range(KH)]
            vd0 = v0 + d0

            def ev_o0(m, nsl, psum):
                nc.scalar.activation(out0[m][:, nsl], psum[:], AF.Tanh,
                                     bias=pp_sb["bout0"][:, m:m + 1])
            mm_phase(wout0_k, K2H, vd0, ev_o0)
            if dbg:
                for j in range(KH):
                    nc.sync.dma_start(dbg_k["O0d"][j], out0[j][:].bitcast(F32))

            # ---------------- layer 1 ----------------
            v1 = [apool.tile([128, T], F32, name=f"v1_{j}", tag="v") for j in range(KH)]
            d1 = [apool.tile([128, T], F32R, name=f"d1_{j}", tag="d") for j in range(KH)]

            def ev_p1(m, nsl, psum):
                nc.scalar.activation(v1[m][:, nsl], psum[:], AF.Identity,
                                     bias=pp_sb["bin1"][:, m:m + 1])
            mm_phase(win1_k, KH, out0, ev_p1)
            v1r = [apool.tile([128, T], F32R, name=f"v1r_{j}", tag="vr")
                   for j in range(KH)]
            scan_phase(v1, d1, pp_sb["alpha1"], pp_sb["thr1"],
                       *((dbg_k["P1d"], dbg_k["V1d"], dbg_k["D1d"]) if dbg else (None, None, None)),
                       vr_tiles=v1r)

            out1 = [apool.tile([128, T], F32R, name=f"o1_{j}", tag="o") for j in range(KH)]
            vd1 = [t[:] for t in v1r] + [t[:] for t in d1]

            def ev_o1(m, nsl, psum):
                nc.scalar.activation(out1[m][:, nsl], psum[:], AF.Tanh,
                                     bias=pp_sb["bout1"][:, m:m + 1])
            mm_phase(wout1_k, K2H, vd1, ev_o1, wdt=F32R)
            if dbg:
                for j in range(KH):
                    nc.sync.dma_start(dbg_k["O1d"][j], out1[j][:].bitcast(F32))

            # ---------------- lm_head ----------------
            # wh chunks / output staging reuse the (now dead) v/d tag slots.
            for n in range(NVC):
                vsl = slice(n * VCH, (n + 1) * VCH)
                whq = []
                for q in range(4):
                    w2 = apool.tile([128, 2 * VCH], F32R, name=f"wh{n}_{q}", tag="v")
                    for kk in range(2):
                        nc.sync.dma_start(w2[:, kk * VCH:(kk + 1) * VCH],
                                          whead_k[2 * q + kk][:, vsl])
                    whq.append(w2)
                bh = spool.tile([1, VCH], F32, name=f"bh{n}", tag="bh", bufs=2)
                nc.sync.dma_start(bh[:], bhead[:, vsl])
                bc = spool.tile([128, VCH], F32, name=f"bc{n}", tag="bc", bufs=2)
                nc.gpsimd.partition_broadcast(bc[:], bh[:])
                for m in range(KH):
                    pl = pspool.tile([128, VCH], F32, name=f"pl{n}_{m}", tag="p")
                    for k in range(KH):
                        nc.tensor.matmul(
                            pl[:],
                            out1[k][:, m * 128:(m + 1) * 128],
                            whq[k // 2][:, (k % 2) * VCH:(k % 2 + 1) * VCH],
                            start=(k == 0), stop=(k == KH - 1))
                    ot = apool.tile([128, VCH], F32, name=f"ot{n}_{m}", tag="d")
                    nc.vector.tensor_add(ot[:], pl[:], bc[:])
                    nc.sync.dma_start(logits[m * 128:(m + 1) * 128, vsl], ot[:])

    nc.finalize()
    return nc


def kernel(x, w_in_0, b_in_0, tau_0, thr_0, w_out_0, b_out_0,
           w_in_1, b_in_1, tau_1, thr_1, w_out_1, b_out_1, w_head, b_head):
    if "nc" not in _cached:
        _cached["nc"] = _build_program()
    nc = _cached["nc"]

    def pp(vec):
        return np.ascontiguousarray(
            np.asarray(vec, dtype=np.float32).reshape(KH, 128).T)

    alpha0 = 1.0 / (1.0 + np.exp(np.asarray(tau_0, dtype=np.float64)))
    alpha1 = 1.0 / (1.0 + np.exp(np.asarray(tau_1, dtype=np.float64)))

    common = {
        "w_in_0": np.ascontiguousarray(w_in_0, dtype=np.float32),
        "w_out_0": np.ascontiguousarray(w_out_0, dtype=np.float32),
        "w_in_1": np.ascontiguousarray(w_in_1, dtype=np.float32),
        "w_out_1": np.ascontiguousarray(w_out_1, dtype=np.float32),
        "alpha0": pp(alpha0), "thr0": pp(thr_0),
        "bin0": pp(b_in_0), "bout0": pp(b_out_0),
        "alpha1": pp(alpha1), "thr1": pp(thr_1),
        "bin1": pp(b_in_1), "bout1": pp(b_out_1),
    }
    wh_half = [np.ascontiguousarray(w_head[:, v * VSH:(v + 1) * VSH], dtype=np.float32)
               for v in range(2)]
    bh_half = [np.ascontiguousarray(b_head[v * VSH:(v + 1) * VSH],
                                    dtype=np.float32).reshape(1, VSH)
               for v in range(2)]
    xT_b = [np.ascontiguousarray(np.asarray(x[b], dtype=np.float32).T)
            for b in range(B)]

    in_maps = []
    for c in range(NCORE):
        b, v = c // 2, c % 2
        in_maps.append({**common, "xT": xT_b[b],
                        "whead": wh_half[v], "bhead": bh_half[v]})

    trace = os.environ.get("KTRACE") == "1"
    res = run_bass_kernel_spmd(nc, in_maps, list(range(NCORE)), trace=trace)
    _cached["res"] = res

    out = np.empty((B, S, V), dtype=np.float32)
    for c in range(NCORE):
        b, v = c // 2, c % 2
        out[b, :, v * VSH:(v + 1) * VSH] = res.results[c]["logits"]
    return out
